# revision 25
# baseline (speedup 1.0000x reference)
"""Trainium2 Bass kernel for DeepFunnelTransactionMLP.

MLP funnel 15->30->60->90->120->90->60->30->15->10->5->1 (ReLU between,
sigmoid at the end) over a batch of 524288 rows, fp32.

Strategy
--------
Pure data parallel: 8 cores x 65536 rows. On each core, activations are
kept feature-major (features on SBUF partitions, batch streaming on the
free dim), so every layer is one (or two) matmul(s) with the weight as
the stationary operand. Small layers are packed block-diagonally: e.g.
layer1 (15->30) processes 4 independent batch chunks in a single matmul
(4x15 input rows -> 4x30 output rows). Bias+ReLU are fused into single
ScalarE activation / VectorE tensor_scalar instructions reading PSUM;
those two engines are the throughput bottleneck, so the tail layers are
packed to the partition limit (L10 out 16-packed, L11 out 64-packed in
bf16) to minimize drained columns.

The batch is processed as a sequence of tiles running a 3-stage software
pipeline (L1-L3 | L4-L5 | L6-L11); the last tiles shrink (1024/512/256/
256 input-cols) so the pipeline flush costs less.

Host side does the free work: transposing/packing x, building the
block-diagonal weights, and unpermuting the output.
"""

import os
import sys

# The bass PJRT path needs the axon jax platform; undo a cpu-only pin if one
# is set (harmless when jax was already imported by the caller).
if os.environ.get("JAX_PLATFORMS") not in (None, "", "axon", "axon,cpu"):
    os.environ["JAX_PLATFORMS"] = ""

sys.path.insert(0, "/opt/trn_rl_repo")

import numpy as np

import concourse.bacc as bacc
import concourse.mybir as mybir
from concourse.bass_utils import run_bass_kernel_spmd
from concourse.tile import TileContext

_DIMS = [15, 30, 60, 90, 120, 90, 60, 30, 15, 10, 5, 1]
NCORES = 8
B = 524288
BC = B // NCORES  # 65536 rows per core
F32 = mybir.dt.float32
F32R = mybir.dt.float32r
BF16 = mybir.dt.bfloat16

# Virtual tiles: widths in xt input columns (each covers 4*W batch rows).
# W=512 would be the floor (below that f32r matmuls drop under 256 moving
# columns -> 4 cyc/row), but uniform 1024 tiles measure best: the pipeline
# flush is latency-bound, so smaller tail tiles only add per-instruction
# drain overhead.
_TILE_WS = [1024] * 16
assert sum(_TILE_WS) == BC // 4
_TILE_X0 = []
_acc = 0
for _w in _TILE_WS:
    _TILE_X0.append(_acc)
    _acc += _w

# Weight variants: (layer l (1-based), K, M, [(koff, moff), ...]).
# lhsT[koff+k, moff+m] = W_l[m, k] for each block; matmul out = lhsT.T @ rhs.
_VARIANTS = [
    ("w1", 1, 60, 120, [(15 * j, 30 * j) for j in range(4)]),
    ("w2A", 2, 60, 120, [(0, 0), (30, 60)]),
    ("w2B", 2, 120, 120, [(60, 0), (90, 60)]),
    ("w3A", 3, 60, 90, [(0, 0)]),
    ("w3B", 3, 120, 90, [(60, 0)]),
    ("w4", 4, 90, 120, [(0, 0)]),
    ("w5", 5, 120, 90, [(0, 0)]),
    ("w6A", 6, 90, 60, [(0, 0)]),
    ("w6B", 6, 90, 120, [(0, 60)]),
    ("w7A", 7, 120, 60, [(0, 0), (60, 30)]),
    ("w7B", 7, 120, 120, [(0, 60), (60, 90)]),
    ("w8A", 8, 120, 60, [(30 * j, 15 * j) for j in range(4)]),
    ("w8B", 8, 120, 120, [(30 * j, 60 + 15 * j) for j in range(4)]),
    ("w9", 9, 120, 80, [(15 * j, 10 * j) for j in range(8)]),
    # L10 output 16-packed: out col c' <- in cols {2c', 2c'+1}; variant m
    # handles in col parity m, writing out partition block 40m:40m+40.
    ("w10A", 10, 80, 80, [(10 * j, 5 * j) for j in range(8)]),
    ("w10B", 10, 80, 80, [(10 * j, 40 + 5 * j) for j in range(8)]),
]
# L11 is bf16 (its matmuls have N < 256, where f32r drops to 4 cyc/row):
# out col c'' <- in cols {4c''+m}; variant m maps in row i -> out partition
# 16m+i. Packed in its own [80, 256] bf16 tile, variant m at cols 64m:64m+64.
_W11_VARIANTS = [(m, [(5 * i, 16 * m + i) for i in range(16)]) for m in range(4)]

_VIDX = {name: i for i, (name, *_) in enumerate(_VARIANTS)}
# tight column packing: variant i starts at the cumulative sum of M widths
_WOFF = {}
_wc = 0
for name, _, _, M, _ in _VARIANTS:
    _WOFF[name] = _wc
    _wc += M
W_COLS = _wc

# Bias layouts: (layer l, tile count) -> packed [tile*dim] at column l-1.
_BIAS_TILES = [4, 2, 1, 1, 1, 2, 4, 8, 8, 16, 64]


def _pack_weights(Ws):
    w = np.zeros((128, W_COLS), dtype=np.float32)
    for name, l, K, M, blocks in _VARIANTS:
        Wl = Ws[l - 1]  # [fan_out, fan_in]
        fo, fi = Wl.shape
        c0 = _WOFF[name]
        for koff, moff in blocks:
            w[koff : koff + fi, c0 + moff : c0 + moff + fo] = Wl.T
    return w


def _pack_w11(W11):
    import ml_dtypes

    w = np.zeros((80, 256), dtype=np.float32)
    for m, blocks in _W11_VARIANTS:
        for koff, moff in blocks:
            w[koff : koff + 5, 64 * m + moff] = W11[0, :]
    return w.astype(ml_dtypes.bfloat16)


def _pack_biases(bs):
    b = np.zeros((128, 16), dtype=np.float32)
    for l, (bl, nt) in enumerate(zip(bs, _BIAS_TILES)):
        v = np.tile(bl, nt)
        b[: v.shape[0], l] = v
    return b


def _out_map(W):
    """batch-row (within a width-W virtual tile, 4W rows) for output element
    [partition p, col c''], c'' in [0, W/16).

    y[p, y0 + c''] = row v((W/2)*(i%8) + 2*(4*c''+m) + i//8) with m=p//16,
    i=p%16, where v swaps the middle two W-blocks (the h2/h3 column
    permutation).
    """
    p = np.arange(64)[:, None]
    cpp = np.arange(W // 16)[None, :]
    m, i = p // 16, p % 16
    cprime = 4 * cpp + m
    c = (W // 2) * (i % 8) + 2 * cprime + i // 8
    v = c.copy()
    v[(c >= W) & (c < 2 * W)] += W
    v[(c >= 2 * W) & (c < 3 * W)] -= W
    return v


_NC_CACHE = None


def _build_nc():
    global _NC_CACHE
    if _NC_CACHE is not None:
        return _NC_CACHE

    nc = bacc.Bacc("TRN2", target_bir_lowering=False, debug=False, num_devices=NCORES)
    xt = nc.dram_tensor("xt", [60, BC // 4], F32R, kind="ExternalInput")
    wd = nc.dram_tensor("w", [128, W_COLS], F32R, kind="ExternalInput")
    w11d = nc.dram_tensor("w11", [80, 256], BF16, kind="ExternalInput")
    bd = nc.dram_tensor("b", [128, 16], F32, kind="ExternalInput")
    y = nc.dram_tensor("y", [64, BC // 64], F32, kind="ExternalOutput")

    NT = len(_TILE_WS)

    with TileContext(nc) as tc:
        with (
            tc.tile_pool(name="const", bufs=1) as cpool,
            tc.tile_pool(name="act", bufs=1) as apool,
            tc.tile_pool(name="act2", bufs=2) as apool2,
            tc.tile_pool(name="io", bufs=3) as iopool,
            tc.tile_pool(name="psum", bufs=4, space="PSUM") as pspool,
        ):
            wsb = cpool.tile([128, W_COLS], F32R, tag="w")
            w11sb = cpool.tile([80, 256], BF16, tag="w11")
            bsb = cpool.tile([128, 16], F32, tag="b")

            # Startup DMA order: w1 slice, x tile 0, biases, then the
            # bulk of the weights (first needed at the first L2 round).
            nc.sync.dma_start(out=wsb[:, 0:120], in_=wd[:, 0:120])
            h0_0 = iopool.tile([60, 1024], F32R, tag="h0", name="h0_0")
            nc.sync.dma_start(out=h0_0[:], in_=xt[:, 0:1024])
            nc.sync.dma_start(out=bsb[:], in_=bd[:])
            nc.sync.dma_start(out=wsb[:, 120:W_COLS], in_=wd[:, 120:W_COLS])
            nc.sync.dma_start(out=w11sb[:], in_=w11d[:])

            # Dummy sigmoid first: loads the sigmoid_and_others table set
            # (which also serves Relu) once during startup, instead of a
            # ~1.3us mid-pipeline table switch at the first real sigmoid.
            scr = cpool.tile([1, 1], F32, tag="scr", name="scr")
            nc.vector.memset(scr[:], 0.0)
            nc.scalar.activation(scr[:], scr[:],
                                 mybir.ActivationFunctionType.Sigmoid,
                                 bias=0.0, scale=1.0)

            def w_ap(name):
                _, _, K, M, _ = _VARIANTS[_VIDX[name]]
                c0 = _WOFF[name]
                return wsb[0:K, c0 : c0 + M]

            def b_ap(l, P):
                return bsb[0:P, l - 1 : l]

            # modeled cumulative busy ns on [ScalarE, VectorE]; ScalarE
            # pre-charged for the act-table load
            sched = {"pe": 0.0, "eng": [1400.0, 0.0]}

            def drain(ps, h_slice, l, P, cols, force_scalar=False, split=False):
                """Fused bias+relu psum->sbuf on the less-loaded act engine;
                split=True halves it across both engines (lower latency,
                extra per-inst overhead — only for the pipeline flush)."""
                if split and cols >= 512 and not force_scalar:
                    cA = (int(cols * 5 / 9) + 15) // 16 * 16
                    nc.scalar.activation(
                        h_slice[:, 0:cA], ps[:, 0:cA],
                        mybir.ActivationFunctionType.Relu,
                        bias=b_ap(l, P), scale=1.0,
                    )
                    sched["eng"][0] += cA / 1.2 + 185
                    nc.vector.tensor_scalar(
                        out=h_slice[:, cA:cols], in0=ps[:, cA:cols],
                        scalar1=b_ap(l, P), scalar2=0.0,
                        op0=mybir.AluOpType.add, op1=mybir.AluOpType.max,
                    )
                    sched["eng"][1] += (cols - cA) / 0.96 + 125
                    return
                if force_scalar or sched["eng"][0] + cols / 1.2 + 185 <= sched["eng"][1] + cols / 0.96 + 125:
                    nc.scalar.activation(
                        h_slice, ps, mybir.ActivationFunctionType.Relu,
                        bias=b_ap(l, P), scale=1.0,
                    )
                    sched["eng"][0] += cols / 1.2 + 185
                else:
                    nc.vector.tensor_scalar(
                        out=h_slice, in0=ps,
                        scalar1=b_ap(l, P), scalar2=0.0,
                        op0=mybir.AluOpType.add, op1=mybir.AluOpType.max,
                    )
                    sched["eng"][1] += cols / 0.96 + 125

            def round_(mms, h_slice, l, P, cols, ps_ext=None, force_scalar=False,
                       split=False):
                """One psum tile: a sequence of matmul groups (each group =
                one start=True..stop=True accumulation over the same psum
                columns; column cursor advances by the group's rhs width on
                stop), then the drain. ps_ext=(tile, col0) reuses a shared
                psum tile at the given column offset."""
                if ps_ext is None:
                    ps = pspool.tile([128, 1024], F32, tag="ps", name="ps")
                    col0 = 0
                else:
                    ps, col0 = ps_ext
                cur = col0
                base = col0
                for wname, rhs, start, stop in mms:
                    _, _, K, M, _ = _VARIANTS[_VIDX[wname]]
                    n = rhs.shape[-1]
                    if start:
                        base = cur
                    nc.tensor.matmul(ps[0:M, base : base + n], w_ap(wname),
                                     rhs, start=start, stop=stop)
                    sched["pe"] += n * 0.4167 + 2.2
                    if stop:
                        cur = base + n
                drain(ps[0:P, col0 : col0 + cols], h_slice, l, P, cols,
                      force_scalar=force_scalar, split=split)

            def build_t1(ti, d):
                """L1-L3: 7 rounds."""
                x0, W = _TILE_X0[ti], _TILE_WS[ti]
                H = W // 2
                spl = False
                rounds = []

                def r_dma():
                    if ti == 0:
                        d["h0"] = h0_0
                    else:
                        d["h0"] = iopool.tile([60, W], F32R, tag="h0",
                                              name=f"h0_{ti}")
                        nc.sync.dma_start(out=d["h0"][:], in_=xt[:, x0 : x0 + W])
                    d["h1"] = apool2.tile([120, W], F32R, tag="h1", name="h1")
                    round_([("w1", d["h0"][0:60, 0:H], True, True),
                            ("w1", d["h0"][0:60, H:W], True, True)],
                           d["h1"][:, :], 1, 120, W, split=spl)
                rounds.append(r_dma)

                def r_l2(half):
                    def f():
                        if half == 0:
                            d["h2"] = apool2.tile([120, 2 * W], F32R, tag="h2", name="h2")
                            round_([("w2A", d["h1"][0:60, 0:H], True, True),
                                    ("w2A", d["h1"][0:60, H:W], True, True)],
                                   d["h2"][:, 0:W], 2, 120, W, split=spl)
                        else:
                            round_([("w2B", d["h1"][0:120, 0:H], True, True),
                                    ("w2B", d["h1"][0:120, H:W], True, True)],
                                   d["h2"][:, W : 2 * W], 2, 120, W, split=spl)
                    return f
                rounds += [r_l2(0), r_l2(1)]

                def r_l3(r):
                    def f():
                        if r == 0:
                            d["h3"] = apool2.tile([90, 4 * W], F32R, tag="h3", name="h3")
                        if r < 2:
                            round_([("w3A", d["h2"][0:60, W * r : W * r + H], True, True),
                                    ("w3A", d["h2"][0:60, W * r + H : W * (r + 1)], True, True)],
                                   d["h3"][:, W * r : W * (r + 1)], 3, 90, W, split=spl)
                        else:
                            rr = r - 2
                            round_([("w3B", d["h2"][0:120, W * rr : W * rr + H], True, True),
                                    ("w3B", d["h2"][0:120, W * rr + H : W * (rr + 1)], True, True)],
                                   d["h3"][:, 2 * W + W * rr : 2 * W + W * (rr + 1)], 3, 90, W, split=spl)
                    return f
                rounds += [r_l3(r) for r in range(4)]
                return rounds

            def build_t2(ti, d):
                """L4-L5: 8 rounds."""
                W = _TILE_WS[ti]
                H = W // 2
                spl = False
                rounds = []

                def r_l4(r):
                    def f():
                        if r == 0:
                            d["h4"] = apool.tile([120, 4 * W], F32R, tag="h4", name="h4")
                        round_([("w4", d["h3"][0:90, W * r : W * r + H], True, True),
                                ("w4", d["h3"][0:90, W * r + H : W * (r + 1)], True, True)],
                               d["h4"][:, W * r : W * (r + 1)], 4, 120, W, split=spl)
                    return f
                rounds += [r_l4(r) for r in range(4)]

                def r_l5(r):
                    def f():
                        if r == 0:
                            d["h5"] = apool2.tile([90, 4 * W], F32R, tag="h5", name="h5")
                        round_([("w5", d["h4"][0:120, W * r : W * r + H], True, True),
                                ("w5", d["h4"][0:120, W * r + H : W * (r + 1)], True, True)],
                               d["h5"][:, W * r : W * (r + 1)], 5, 90, W, split=spl)
                    return f
                rounds += [r_l5(r) for r in range(4)]
                return rounds

            def build_t3(ti, d):
                """L6-L11: 7 rounds."""
                x0, W = _TILE_X0[ti], _TILE_WS[ti]
                H = W // 2
                spl = ti == NT - 1  # halve flush drain latency
                rounds = []

                def r_l6(r):
                    def f():
                        if r == 0:
                            d["h6"] = apool2.tile([120, 2 * W], F32R, tag="h6", name="h6")
                        mms = []
                        for q in range(2):
                            w = 2 * r + q
                            mms.append(("w6A", d["h5"][0:90, W * w : W * w + H], True, False))
                            mms.append(("w6B", d["h5"][0:90, W * w + H : W * (w + 1)], False, True))
                        round_(mms, d["h6"][:, W * r : W * (r + 1)], 6, 120, W, split=spl)
                    return f
                rounds += [r_l6(r) for r in range(2)]

                def r_l7():
                    d["h7"] = apool2.tile([120, W], F32R, tag="h7", name="h7")
                    mms = []
                    for w in range(2):
                        mms.append(("w7A", d["h6"][0:120, W * w : W * w + H], True, False))
                        mms.append(("w7B", d["h6"][0:120, W * w + H : W * (w + 1)], False, True))
                    round_(mms, d["h7"][:, :], 7, 120, W, split=spl)
                rounds.append(r_l7)

                def r_l8():
                    d["h8"] = apool2.tile([120, H], F32R, tag="h8", name="h8")
                    d["pstail"] = pspool.tile([128, 1024], F32, tag="ps", name="pstail")
                    round_([("w8A", d["h7"][0:120, 0:H], True, False),
                            ("w8B", d["h7"][0:120, H:W], False, True)],
                           d["h8"][:, :], 8, 120, H, ps_ext=(d["pstail"], 0))
                rounds.append(r_l8)

                def r_l9():
                    d["h9"] = apool2.tile([80, H], F32R, tag="h9", name="h9")
                    round_([("w9", d["h8"][0:120, :], True, True)], d["h9"][:, :],
                           9, 80, H, ps_ext=(d["pstail"], H))
                rounds.append(r_l9)

                def r_l10():
                    d["h10"] = apool2.tile([80, W // 4], BF16, tag="h10", name="h10")
                    d["pstail2"] = pspool.tile([128, 1024], F32, tag="ps", name="pstail2")
                    round_([("w10A", d["h9"][0:80, 0::2], True, False),
                            ("w10B", d["h9"][0:80, 1::2], False, True)],
                           d["h10"][:, :], 10, 80, W // 4, ps_ext=(d["pstail2"], 0))
                rounds.append(r_l10)

                def r_l11():
                    C = W // 16
                    ps = d["pstail2"]
                    # own bank (cols 512+): sharing a psum bank with L10's
                    # accumulation region races with L10's drain
                    for m in range(4):
                        nc.tensor.matmul(ps[0:64, 512 : 512 + C],
                                         w11sb[0:80, 64 * m : 64 * (m + 1)],
                                         d["h10"][0:80, m::4],
                                         start=(m == 0), stop=(m == 3))
                        sched["pe"] += C * 0.4167 + 2.2
                    osb = iopool.tile([64, C], F32, tag="osb", name="osb")
                    nc.scalar.activation(
                        osb[:, :], ps[0:64, 512 : 512 + C],
                        mybir.ActivationFunctionType.Sigmoid,
                        bias=b_ap(11, 64), scale=1.0,
                    )
                    sched["eng"][0] += C / 1.2 + 185
                    y0 = x0 // 16
                    nc.sync.dma_start(out=y[:, y0 : y0 + C], in_=osb[:, :])
                rounds.append(r_l11)
                return rounds

            # 3-way software pipeline: epoch e runs (L1-L3)(e), (L4-L5)(e-1),
            # (L6-L11)(e-2) round-robin, so three tiles' serial layer chains
            # overlap in every engine's in-order queue.
            from itertools import zip_longest

            dicts = [dict() for _ in range(NT)]
            for e in range(NT + 2):
                parts = []
                if e < NT:
                    parts.append(build_t1(e, dicts[e]))
                if 0 <= e - 1 < NT:
                    parts.append(build_t2(e - 1, dicts[e - 1]))
                if 0 <= e - 2 < NT:
                    parts.append(build_t3(e - 2, dicts[e - 2]))
                for grp in zip_longest(*parts):
                    for r in grp:
                        if r is not None:
                            r()

    nc.compile()
    _NC_CACHE = nc
    return nc


def _make_in_maps(inputs):
    x = np.asarray(inputs["x"], dtype=np.float32)
    Ws = [np.asarray(inputs[f"W{i}"], dtype=np.float32) for i in range(1, 12)]
    bs = [np.asarray(inputs[f"b{i}"], dtype=np.float32) for i in range(1, 12)]

    w_pack = _pack_weights(Ws)
    w11_pack = _pack_w11(Ws[10])
    b_pack = _pack_biases(bs)

    in_maps = []
    for c in range(NCORES):
        xc = x[c * BC : (c + 1) * BC]
        # per virtual tile: xt[15j+f, x0+m] = xc[4*x0 + j*W + m, f]
        xt = np.empty((60, BC // 4), dtype=np.float32)
        for x0, W in zip(_TILE_X0, _TILE_WS):
            blk = xc[4 * x0 : 4 * (x0 + W)].reshape(4, W, _DIMS[0])
            xt[:, x0 : x0 + W] = blk.transpose(0, 2, 1).reshape(60, W)
        in_maps.append({"xt": xt, "w": w_pack, "w11": w11_pack, "b": b_pack})
    return in_maps


def kernel(**inputs):
    in_maps = _make_in_maps(inputs)
    nc = _build_nc()
    res = run_bass_kernel_spmd(nc, in_maps, list(range(NCORES)))

    omaps = {W: _out_map(W) for W in set(_TILE_WS)}
    out = np.empty((B, 1), dtype=np.float32)
    for c in range(NCORES):
        yc = res.results[c]["y"]  # [64, BC//64]
        for x0, W in zip(_TILE_X0, _TILE_WS):
            y0 = x0 // 16
            blk = np.empty(4 * W, dtype=np.float32)
            blk[omaps[W].ravel()] = yc[:, y0 : y0 + W // 16].ravel()
            out[c * BC + 4 * x0 : c * BC + 4 * (x0 + W), 0] = blk
    return out


# revision 26
# speedup vs baseline: 1.0013x; 1.0013x over previous
"""Trainium2 Bass kernel for DeepFunnelTransactionMLP.

MLP funnel 15->30->60->90->120->90->60->30->15->10->5->1 (ReLU between,
sigmoid at the end) over a batch of 524288 rows, fp32.

Strategy
--------
Pure data parallel: 8 cores x 65536 rows. On each core, activations are
kept feature-major (features on SBUF partitions, batch streaming on the
free dim), so every layer is one (or two) matmul(s) with the weight as
the stationary operand. Small layers are packed block-diagonally: e.g.
layer1 (15->30) processes 4 independent batch chunks in a single matmul
(4x15 input rows -> 4x30 output rows). Bias+ReLU are fused into single
ScalarE activation / VectorE tensor_scalar instructions reading PSUM;
those two engines are the throughput bottleneck, so the tail layers are
packed to the partition limit (L10 out 16-packed, L11 out 64-packed in
bf16) to minimize drained columns.

The batch is processed as a sequence of tiles running a 3-stage software
pipeline (L1-L3 | L4-L5 | L6-L11); the last tiles shrink (1024/512/256/
256 input-cols) so the pipeline flush costs less.

Host side does the free work: transposing/packing x, building the
block-diagonal weights, and unpermuting the output.
"""

import os
import sys

# The bass PJRT path needs the axon jax platform; undo a cpu-only pin if one
# is set (harmless when jax was already imported by the caller).
if os.environ.get("JAX_PLATFORMS") not in (None, "", "axon", "axon,cpu"):
    os.environ["JAX_PLATFORMS"] = ""

sys.path.insert(0, "/opt/trn_rl_repo")

import numpy as np

import concourse.bacc as bacc
import concourse.mybir as mybir
from concourse.bass_utils import run_bass_kernel_spmd
from concourse.tile import TileContext

_DIMS = [15, 30, 60, 90, 120, 90, 60, 30, 15, 10, 5, 1]
NCORES = 8
B = 524288
BC = B // NCORES  # 65536 rows per core
F32 = mybir.dt.float32
F32R = mybir.dt.float32r
BF16 = mybir.dt.bfloat16

# Virtual tiles: widths in xt input columns (each covers 4*W batch rows).
# W=512 would be the floor (below that f32r matmuls drop under 256 moving
# columns -> 4 cyc/row), but uniform 1024 tiles measure best: the pipeline
# flush is latency-bound, so smaller tail tiles only add per-instruction
# drain overhead.
_TILE_WS = [1024] * 16
assert sum(_TILE_WS) == BC // 4
_TILE_X0 = []
_acc = 0
for _w in _TILE_WS:
    _TILE_X0.append(_acc)
    _acc += _w

# Weight variants: (layer l (1-based), K, M, [(koff, moff), ...]).
# lhsT[koff+k, moff+m] = W_l[m, k] for each block; matmul out = lhsT.T @ rhs.
_VARIANTS = [
    ("w1", 1, 60, 120, [(15 * j, 30 * j) for j in range(4)]),
    ("w2A", 2, 60, 120, [(0, 0), (30, 60)]),
    ("w2B", 2, 120, 120, [(60, 0), (90, 60)]),
    ("w3A", 3, 60, 90, [(0, 0)]),
    ("w3B", 3, 120, 90, [(60, 0)]),
    ("w4", 4, 90, 120, [(0, 0)]),
    ("w5", 5, 120, 90, [(0, 0)]),
    ("w6A", 6, 90, 60, [(0, 0)]),
    ("w6B", 6, 90, 120, [(0, 60)]),
    ("w7A", 7, 120, 60, [(0, 0), (60, 30)]),
    ("w7B", 7, 120, 120, [(0, 60), (60, 90)]),
    ("w8A", 8, 120, 60, [(30 * j, 15 * j) for j in range(4)]),
    ("w8B", 8, 120, 120, [(30 * j, 60 + 15 * j) for j in range(4)]),
    ("w9", 9, 120, 80, [(15 * j, 10 * j) for j in range(8)]),
    # L10 output 16-packed: out col c' <- in cols {2c', 2c'+1}; variant m
    # handles in col parity m, writing out partition block 40m:40m+40.
    ("w10A", 10, 80, 80, [(10 * j, 5 * j) for j in range(8)]),
    ("w10B", 10, 80, 80, [(10 * j, 40 + 5 * j) for j in range(8)]),
]
# L11 is bf16 (its matmuls have N < 256, where f32r drops to 4 cyc/row):
# out col c'' <- in cols {4c''+m}; variant m maps in row i -> out partition
# 16m+i. Packed in its own [80, 256] bf16 tile, variant m at cols 64m:64m+64.
_W11_VARIANTS = [(m, [(5 * i, 16 * m + i) for i in range(16)]) for m in range(4)]

_VIDX = {name: i for i, (name, *_) in enumerate(_VARIANTS)}
# tight column packing: variant i starts at the cumulative sum of M widths
_WOFF = {}
_wc = 0
for name, _, _, M, _ in _VARIANTS:
    _WOFF[name] = _wc
    _wc += M
W_COLS = _wc

# Bias layouts: (layer l, tile count) -> packed [tile*dim] at column l-1.
_BIAS_TILES = [4, 2, 1, 1, 1, 2, 4, 8, 8, 16, 64]


def _pack_weights(Ws):
    w = np.zeros((128, W_COLS), dtype=np.float32)
    for name, l, K, M, blocks in _VARIANTS:
        Wl = Ws[l - 1]  # [fan_out, fan_in]
        fo, fi = Wl.shape
        c0 = _WOFF[name]
        for koff, moff in blocks:
            w[koff : koff + fi, c0 + moff : c0 + moff + fo] = Wl.T
    return w


def _pack_w11(W11):
    import ml_dtypes

    w = np.zeros((80, 256), dtype=np.float32)
    for m, blocks in _W11_VARIANTS:
        for koff, moff in blocks:
            w[koff : koff + 5, 64 * m + moff] = W11[0, :]
    return w.astype(ml_dtypes.bfloat16)


def _pack_biases(bs):
    b = np.zeros((128, 16), dtype=np.float32)
    for l, (bl, nt) in enumerate(zip(bs, _BIAS_TILES)):
        v = np.tile(bl, nt)
        b[: v.shape[0], l] = v
    return b


def _out_map(W):
    """batch-row (within a width-W virtual tile, 4W rows) for output element
    [partition p, col c''], c'' in [0, W/16).

    y[p, y0 + c''] = row v((W/2)*(i%8) + 2*(4*c''+m) + i//8) with m=p//16,
    i=p%16, where v swaps the middle two W-blocks (the h2/h3 column
    permutation).
    """
    p = np.arange(64)[:, None]
    cpp = np.arange(W // 16)[None, :]
    m, i = p // 16, p % 16
    cprime = 4 * cpp + m
    c = (W // 2) * (i % 8) + 2 * cprime + i // 8
    v = c.copy()
    v[(c >= W) & (c < 2 * W)] += W
    v[(c >= 2 * W) & (c < 3 * W)] -= W
    return v


_NC_CACHE = None


def _build_nc():
    global _NC_CACHE
    if _NC_CACHE is not None:
        return _NC_CACHE

    nc = bacc.Bacc("TRN2", target_bir_lowering=False, debug=False, num_devices=NCORES)
    xt = nc.dram_tensor("xt", [60, BC // 4], F32R, kind="ExternalInput")
    wd = nc.dram_tensor("w", [128, W_COLS], F32R, kind="ExternalInput")
    w11d = nc.dram_tensor("w11", [80, 256], BF16, kind="ExternalInput")
    bd = nc.dram_tensor("b", [128, 16], F32, kind="ExternalInput")
    y = nc.dram_tensor("y", [64, BC // 64], F32, kind="ExternalOutput")

    NT = len(_TILE_WS)

    with TileContext(nc) as tc:
        with (
            tc.tile_pool(name="const", bufs=1) as cpool,
            tc.tile_pool(name="act", bufs=1) as apool,
            tc.tile_pool(name="act2", bufs=2) as apool2,
            tc.tile_pool(name="io", bufs=3) as iopool,
            tc.tile_pool(name="psum", bufs=4, space="PSUM") as pspool,
        ):
            wsb = cpool.tile([128, W_COLS], F32R, tag="w")
            w11sb = cpool.tile([80, 256], BF16, tag="w11")
            bsb = cpool.tile([128, 16], F32, tag="b")

            # Startup DMAs split across two queue engines so x tile 0
            # and the L1 weights land in parallel; the bulk of the weights
            # rides behind (first needed at the first L2 round).
            h0_0 = iopool.tile([60, 1024], F32R, tag="h0", name="h0_0")
            nc.sync.dma_start(out=h0_0[:], in_=xt[:, 0:1024])
            nc.scalar.dma_start(out=wsb[:, 0:120], in_=wd[:, 0:120])
            nc.scalar.dma_start(out=bsb[:], in_=bd[:])
            nc.sync.dma_start(out=wsb[:, 120:W_COLS], in_=wd[:, 120:W_COLS])
            nc.sync.dma_start(out=w11sb[:], in_=w11d[:])

            # Dummy sigmoid first: loads the sigmoid_and_others table set
            # (which also serves Relu) once during startup, instead of a
            # ~1.3us mid-pipeline table switch at the first real sigmoid.
            scr = cpool.tile([1, 1], F32, tag="scr", name="scr")
            nc.vector.memset(scr[:], 0.0)
            nc.scalar.activation(scr[:], scr[:],
                                 mybir.ActivationFunctionType.Sigmoid,
                                 bias=0.0, scale=1.0)

            def w_ap(name):
                _, _, K, M, _ = _VARIANTS[_VIDX[name]]
                c0 = _WOFF[name]
                return wsb[0:K, c0 : c0 + M]

            def b_ap(l, P):
                return bsb[0:P, l - 1 : l]

            # modeled cumulative busy ns on [ScalarE, VectorE]; ScalarE
            # pre-charged for the act-table load
            sched = {"pe": 0.0, "eng": [1400.0, 0.0]}

            def drain(ps, h_slice, l, P, cols, force_scalar=False, split=False):
                """Fused bias+relu psum->sbuf on the less-loaded act engine;
                split=True halves it across both engines (lower latency,
                extra per-inst overhead — only for the pipeline flush)."""
                if split and cols >= 512 and not force_scalar:
                    cA = (int(cols * 5 / 9) + 15) // 16 * 16
                    nc.scalar.activation(
                        h_slice[:, 0:cA], ps[:, 0:cA],
                        mybir.ActivationFunctionType.Relu,
                        bias=b_ap(l, P), scale=1.0,
                    )
                    sched["eng"][0] += cA / 1.2 + 185
                    nc.vector.tensor_scalar(
                        out=h_slice[:, cA:cols], in0=ps[:, cA:cols],
                        scalar1=b_ap(l, P), scalar2=0.0,
                        op0=mybir.AluOpType.add, op1=mybir.AluOpType.max,
                    )
                    sched["eng"][1] += (cols - cA) / 0.96 + 125
                    return
                if force_scalar or sched["eng"][0] + cols / 1.2 + 185 <= sched["eng"][1] + cols / 0.96 + 125:
                    nc.scalar.activation(
                        h_slice, ps, mybir.ActivationFunctionType.Relu,
                        bias=b_ap(l, P), scale=1.0,
                    )
                    sched["eng"][0] += cols / 1.2 + 185
                else:
                    nc.vector.tensor_scalar(
                        out=h_slice, in0=ps,
                        scalar1=b_ap(l, P), scalar2=0.0,
                        op0=mybir.AluOpType.add, op1=mybir.AluOpType.max,
                    )
                    sched["eng"][1] += cols / 0.96 + 125

            def round_(mms, h_slice, l, P, cols, ps_ext=None, force_scalar=False,
                       split=False):
                """One psum tile: a sequence of matmul groups (each group =
                one start=True..stop=True accumulation over the same psum
                columns; column cursor advances by the group's rhs width on
                stop), then the drain. ps_ext=(tile, col0) reuses a shared
                psum tile at the given column offset."""
                if ps_ext is None:
                    ps = pspool.tile([128, 1024], F32, tag="ps", name="ps")
                    col0 = 0
                else:
                    ps, col0 = ps_ext
                cur = col0
                base = col0
                for wname, rhs, start, stop in mms:
                    _, _, K, M, _ = _VARIANTS[_VIDX[wname]]
                    n = rhs.shape[-1]
                    if start:
                        base = cur
                    nc.tensor.matmul(ps[0:M, base : base + n], w_ap(wname),
                                     rhs, start=start, stop=stop)
                    sched["pe"] += n * 0.4167 + 2.2
                    if stop:
                        cur = base + n
                drain(ps[0:P, col0 : col0 + cols], h_slice, l, P, cols,
                      force_scalar=force_scalar, split=split)

            def build_t1(ti, d):
                """L1-L3: 7 rounds."""
                x0, W = _TILE_X0[ti], _TILE_WS[ti]
                H = W // 2
                spl = False
                rounds = []

                def r_dma():
                    if ti == 0:
                        d["h0"] = h0_0
                    else:
                        d["h0"] = iopool.tile([60, W], F32R, tag="h0",
                                              name=f"h0_{ti}")
                        nc.sync.dma_start(out=d["h0"][:], in_=xt[:, x0 : x0 + W])
                    d["h1"] = apool2.tile([120, W], F32R, tag="h1", name="h1")
                    round_([("w1", d["h0"][0:60, 0:H], True, True),
                            ("w1", d["h0"][0:60, H:W], True, True)],
                           d["h1"][:, :], 1, 120, W, split=spl)
                rounds.append(r_dma)

                def r_l2(half):
                    def f():
                        if half == 0:
                            d["h2"] = apool2.tile([120, 2 * W], F32R, tag="h2", name="h2")
                            round_([("w2A", d["h1"][0:60, 0:H], True, True),
                                    ("w2A", d["h1"][0:60, H:W], True, True)],
                                   d["h2"][:, 0:W], 2, 120, W, split=spl)
                        else:
                            round_([("w2B", d["h1"][0:120, 0:H], True, True),
                                    ("w2B", d["h1"][0:120, H:W], True, True)],
                                   d["h2"][:, W : 2 * W], 2, 120, W, split=spl)
                    return f
                rounds += [r_l2(0), r_l2(1)]

                def r_l3(r):
                    def f():
                        if r == 0:
                            d["h3"] = apool2.tile([90, 4 * W], F32R, tag="h3", name="h3")
                        if r < 2:
                            round_([("w3A", d["h2"][0:60, W * r : W * r + H], True, True),
                                    ("w3A", d["h2"][0:60, W * r + H : W * (r + 1)], True, True)],
                                   d["h3"][:, W * r : W * (r + 1)], 3, 90, W, split=spl)
                        else:
                            rr = r - 2
                            round_([("w3B", d["h2"][0:120, W * rr : W * rr + H], True, True),
                                    ("w3B", d["h2"][0:120, W * rr + H : W * (rr + 1)], True, True)],
                                   d["h3"][:, 2 * W + W * rr : 2 * W + W * (rr + 1)], 3, 90, W, split=spl)
                    return f
                rounds += [r_l3(r) for r in range(4)]
                return rounds

            def build_t2(ti, d):
                """L4-L5: 8 rounds."""
                W = _TILE_WS[ti]
                H = W // 2
                spl = False
                rounds = []

                def r_l4(r):
                    def f():
                        if r == 0:
                            d["h4"] = apool.tile([120, 4 * W], F32R, tag="h4", name="h4")
                        round_([("w4", d["h3"][0:90, W * r : W * r + H], True, True),
                                ("w4", d["h3"][0:90, W * r + H : W * (r + 1)], True, True)],
                               d["h4"][:, W * r : W * (r + 1)], 4, 120, W, split=spl)
                    return f
                rounds += [r_l4(r) for r in range(4)]

                def r_l5(r):
                    def f():
                        if r == 0:
                            d["h5"] = apool2.tile([90, 4 * W], F32R, tag="h5", name="h5")
                        round_([("w5", d["h4"][0:120, W * r : W * r + H], True, True),
                                ("w5", d["h4"][0:120, W * r + H : W * (r + 1)], True, True)],
                               d["h5"][:, W * r : W * (r + 1)], 5, 90, W, split=spl)
                    return f
                rounds += [r_l5(r) for r in range(4)]
                return rounds

            def build_t3(ti, d):
                """L6-L11: 7 rounds."""
                x0, W = _TILE_X0[ti], _TILE_WS[ti]
                H = W // 2
                spl = False
                rounds = []

                def r_l6(r):
                    def f():
                        if r == 0:
                            d["h6"] = apool2.tile([120, 2 * W], F32R, tag="h6", name="h6")
                        mms = []
                        for q in range(2):
                            w = 2 * r + q
                            mms.append(("w6A", d["h5"][0:90, W * w : W * w + H], True, False))
                            mms.append(("w6B", d["h5"][0:90, W * w + H : W * (w + 1)], False, True))
                        round_(mms, d["h6"][:, W * r : W * (r + 1)], 6, 120, W, split=spl)
                    return f
                rounds += [r_l6(r) for r in range(2)]

                def r_l7():
                    d["h7"] = apool2.tile([120, W], F32R, tag="h7", name="h7")
                    mms = []
                    for w in range(2):
                        mms.append(("w7A", d["h6"][0:120, W * w : W * w + H], True, False))
                        mms.append(("w7B", d["h6"][0:120, W * w + H : W * (w + 1)], False, True))
                    round_(mms, d["h7"][:, :], 7, 120, W, split=spl)
                rounds.append(r_l7)

                def r_l8():
                    d["h8"] = apool2.tile([120, H], F32R, tag="h8", name="h8")
                    d["pstail"] = pspool.tile([128, 1024], F32, tag="ps", name="pstail")
                    round_([("w8A", d["h7"][0:120, 0:H], True, False),
                            ("w8B", d["h7"][0:120, H:W], False, True)],
                           d["h8"][:, :], 8, 120, H, ps_ext=(d["pstail"], 0))
                rounds.append(r_l8)

                def r_l9():
                    d["h9"] = apool2.tile([80, H], F32R, tag="h9", name="h9")
                    round_([("w9", d["h8"][0:120, :], True, True)], d["h9"][:, :],
                           9, 80, H, ps_ext=(d["pstail"], H))
                rounds.append(r_l9)

                def r_l10():
                    d["h10"] = apool2.tile([80, W // 4], BF16, tag="h10", name="h10")
                    d["pstail2"] = pspool.tile([128, 1024], F32, tag="ps", name="pstail2")
                    round_([("w10A", d["h9"][0:80, 0::2], True, False),
                            ("w10B", d["h9"][0:80, 1::2], False, True)],
                           d["h10"][:, :], 10, 80, W // 4, ps_ext=(d["pstail2"], 0))
                rounds.append(r_l10)

                def r_l11():
                    C = W // 16
                    ps = d["pstail2"]
                    # own bank (cols 512+): sharing a psum bank with L10's
                    # accumulation region races with L10's drain
                    for m in range(4):
                        nc.tensor.matmul(ps[0:64, 512 : 512 + C],
                                         w11sb[0:80, 64 * m : 64 * (m + 1)],
                                         d["h10"][0:80, m::4],
                                         start=(m == 0), stop=(m == 3))
                        sched["pe"] += C * 0.4167 + 2.2
                    osb = iopool.tile([64, C], F32, tag="osb", name="osb")
                    nc.scalar.activation(
                        osb[:, :], ps[0:64, 512 : 512 + C],
                        mybir.ActivationFunctionType.Sigmoid,
                        bias=b_ap(11, 64), scale=1.0,
                    )
                    sched["eng"][0] += C / 1.2 + 185
                    y0 = x0 // 16
                    nc.sync.dma_start(out=y[:, y0 : y0 + C], in_=osb[:, :])
                rounds.append(r_l11)
                return rounds

            # 3-way software pipeline: epoch e runs (L1-L3)(e), (L4-L5)(e-1),
            # (L6-L11)(e-2) round-robin, so three tiles' serial layer chains
            # overlap in every engine's in-order queue.
            from itertools import zip_longest

            dicts = [dict() for _ in range(NT)]
            for e in range(NT + 2):
                parts = []
                if e < NT:
                    parts.append(build_t1(e, dicts[e]))
                if 0 <= e - 1 < NT:
                    parts.append(build_t2(e - 1, dicts[e - 1]))
                if 0 <= e - 2 < NT:
                    parts.append(build_t3(e - 2, dicts[e - 2]))
                for grp in zip_longest(*parts):
                    for r in grp:
                        if r is not None:
                            r()

    nc.compile()
    _NC_CACHE = nc
    return nc


def _make_in_maps(inputs):
    x = np.asarray(inputs["x"], dtype=np.float32)
    Ws = [np.asarray(inputs[f"W{i}"], dtype=np.float32) for i in range(1, 12)]
    bs = [np.asarray(inputs[f"b{i}"], dtype=np.float32) for i in range(1, 12)]

    w_pack = _pack_weights(Ws)
    w11_pack = _pack_w11(Ws[10])
    b_pack = _pack_biases(bs)

    in_maps = []
    for c in range(NCORES):
        xc = x[c * BC : (c + 1) * BC]
        # per virtual tile: xt[15j+f, x0+m] = xc[4*x0 + j*W + m, f]
        xt = np.empty((60, BC // 4), dtype=np.float32)
        for x0, W in zip(_TILE_X0, _TILE_WS):
            blk = xc[4 * x0 : 4 * (x0 + W)].reshape(4, W, _DIMS[0])
            xt[:, x0 : x0 + W] = blk.transpose(0, 2, 1).reshape(60, W)
        in_maps.append({"xt": xt, "w": w_pack, "w11": w11_pack, "b": b_pack})
    return in_maps


def kernel(**inputs):
    in_maps = _make_in_maps(inputs)
    nc = _build_nc()
    res = run_bass_kernel_spmd(nc, in_maps, list(range(NCORES)))

    omaps = {W: _out_map(W) for W in set(_TILE_WS)}
    out = np.empty((B, 1), dtype=np.float32)
    for c in range(NCORES):
        yc = res.results[c]["y"]  # [64, BC//64]
        for x0, W in zip(_TILE_X0, _TILE_WS):
            y0 = x0 // 16
            blk = np.empty(4 * W, dtype=np.float32)
            blk[omaps[W].ravel()] = yc[:, y0 : y0 + W // 16].ravel()
            out[c * BC + 4 * x0 : c * BC + 4 * (x0 + W), 0] = blk
    return out
